# revision 25
# baseline (speedup 1.0000x reference)
"""Trainium2 Bass kernel for sparse multi-headed attention (bf16 pipeline).

Semantics (verified against the reference):
  q = x_q @ Wq.T + bq (per head, dk=32), same for k, v
  for each row s: attend to keys {s-c : c in (5,3,1,0), c <= s}
    score_c[s] = q[s].k[s-c] / sqrt(4)
    p = softmax over valid offsets
    attn[s] = sum_c p_c[s] * v[s-c]
  y = attn @ Wo.T + bo

Sharding: data-parallel over d_stock (8 stocks -> 8 cores). Each core
processes 4 (stock,batch) pairs = 2048 rows. Weights replicated.

Device layout: feature-major bf16 activations [256 feats, 2048 rows]
(host pre-transposes + casts). Projections run through [128, 1024]
half-psums (2 banks, ring of 2) so drains pipeline behind the next
half's matmuls, and products/score-matmuls interleave with later
projections. Scores/softmax live in the compact pair-block layout
[128 partitions = 4 pairs x 32, 4 offsets x 512] (fp32 psum -> bf16
sbuf). p is broadcast back to feature partitions with selector
matmuls; the (5,3) halves are drained to SBUF by the scalar engine
(2x DVE muls), the (1,0) halves are multiplied straight out of psum.
Output y is emitted feature-major bf16 and transposed/upcast on host.
k/v tiles carry a 9-col front pad so shifted reads stay 4B-aligned.
"""

import numpy as np
import ml_dtypes

from concourse import bacc, bass, mybir, tile
from concourse.bass_utils import run_bass_kernel_spmd

DS, NB, S, DM, H, DK = 8, 4, 512, 256, 8, 32
CONS = (5, 3, 1, 0)
NCORES = 8
NPAIR = NB  # pairs per core (1 stock x 4 batches)
ROWS = NPAIR * S  # 2048
P = 128
PADK = 9  # front pad columns for k/v (odd -> shifted (5,3,1) reads aligned)
NEG = -1e9
SCALE = 0.5  # 1/sqrt(n_att)

f32 = mybir.dt.float32
bf16 = mybir.dt.bfloat16
Act = mybir.ActivationFunctionType
BF = ml_dtypes.bfloat16


def _emit(ctx, tc, nc, d, y_dram):
    main = ctx.enter_context(tc.tile_pool(name="main", bufs=1))
    prp = ctx.enter_context(tc.tile_pool(name="prp", bufs=3))
    mp = ctx.enter_context(tc.tile_pool(name="mp", bufs=5))
    bsp = ctx.enter_context(tc.tile_pool(name="bsp", bufs=4))
    pjh_cm = tc.tile_pool(name="pjh", bufs=4, space="PSUM")
    pjh = pjh_cm.__enter__()
    scp_cm = tc.tile_pool(name="scp", bufs=1, space="PSUM")
    scp = scp_cm.__enter__()

    # ---------------- PE warmup (HAM un-throttle) while DMAs run ----
    wtile = main.tile([P, 512], bf16, name="wtile")
    nc.vector.memset(wtile[:], 0.0)
    wps = pjh.tile([P, 512], f32, name="pj", tag="pj")
    for i in range(4):
        nc.tensor.matmul(
            wps[:], lhsT=wtile[:, 0:P], rhs=wtile[:], start=True, stop=True)

    # ---------------- loads ----------------
    # sync queue: q-side, v, selectors, weights, y stores.
    # scalar queue: k-side early loads only, then pure ACT work.
    xs = {}
    for name in ("xq", "xk", "xv"):
        for ch in range(2):
            xs[name, ch] = main.tile([P, ROWS], bf16, name=f"{name}{ch}")
    # packed constants: single DMA per group (issue cost ~600ns each,
    # so small tensors ride together)
    wqp = main.tile([P, 512], bf16, name="wqp")
    wkp = main.tile([P, 512], bf16, name="wkp")
    wvop = main.tile([P, 1024], bf16, name="wvop")
    cst = main.tile([P, 8], f32, name="cst")
    skm = main.tile([P, 448], bf16, name="skm")
    smk = main.tile([P, 1024], bf16, name="smk")
    ws = {}
    for ch in range(2):
        ws["wq", ch] = wqp[:, ch * DM:(ch + 1) * DM]
        ws["wk", ch] = wkp[:, ch * DM:(ch + 1) * DM]
        ws["wv", ch] = wvop[:, ch * DM:(ch + 1) * DM]
        ws["wo", ch] = wvop[:, 512 + ch * DM:512 + (ch + 1) * DM]
    bqkv = [cst[:, 0:3], cst[:, 3:6]]
    bo_sb = cst[:, 6:8]
    selkm = [skm[:, 0:224], skm[:, 224:448]]
    selmk = [[smk[:, (2 * p + ch) * P:(2 * p + ch + 1) * P]
              for ch in range(2)] for p in range(NPAIR)]
    nc.sync.dma_start(out=wqp[:], in_=d["wq"])
    nc.scalar.dma_start(out=wkp[:], in_=d["wk"])
    nc.scalar.dma_start(out=cst[:], in_=d["cst"])
    # x loads: pair-0 quarter first (unblocks the first projections),
    # then the remaining 3 pairs in one transfer
    for lo, hi in ((0, 512), (512, 2048)):
        for ch in range(2):
            nc.sync.dma_start(out=xs["xq", ch][:, lo:hi],
                              in_=d["xq"][ch * P:(ch + 1) * P, lo:hi])
            nc.scalar.dma_start(out=xs["xk", ch][:, lo:hi],
                                in_=d["xk"][ch * P:(ch + 1) * P, lo:hi])
    nc.sync.dma_start(out=skm[:], in_=d["selkm"])
    nc.sync.dma_start(out=wvop[:], in_=d["wvo"])
    for lo, hi in ((0, 1024), (1024, 2048)):
        for ch in range(2):
            nc.sync.dma_start(out=xs["xv", ch][:, lo:hi],
                              in_=d["xv"][ch * P:(ch + 1) * P, lo:hi])
    nc.sync.dma_start(out=smk[:], in_=d["selmk"])

    # ---------------- projection tiles ----------------
    # widths kept even so bf16 tiles never knock later SBUF allocations
    # off 4-byte alignment (odd width would disable DVE 2x)
    q_sb = [main.tile([P, ROWS], bf16, name=f"q_sb{ch}") for ch in range(2)]
    k_sb = [main.tile([P, PADK + ROWS + 1], bf16, name=f"k_sb{ch}")
            for ch in range(2)]
    v_sb = [main.tile([P, PADK + ROWS + 1], bf16, name=f"v_sb{ch}")
            for ch in range(2)]
    for ch in range(2):
        nc.vector.memset(k_sb[ch][:, 0:PADK], 0.0)
        nc.vector.memset(v_sb[ch][:, 0:PADK], 0.0)

    def proj_q(name, wname, bcol, ch, qi, out_ap, drain=True):
        """Quarter-projection: rows [512qi, 512qi+512) of output chunk
        ch; ACT drain adds bias and casts bf16."""
        ps = pjh.tile([P, 512], f32, name="pj", tag="pj")
        lo = 512 * qi
        for kch in range(2):
            nc.tensor.matmul(
                ps[:],
                lhsT=ws[wname, kch][:, ch * P:(ch + 1) * P],
                rhs=xs[name, kch][:, lo:lo + 512],
                start=(kch == 0), stop=(kch == 1))
        if drain:
            nc.scalar.activation(
                out_ap, ps[:], Act.Identity, bias=bqkv[ch][:, bcol:bcol + 1])
        return ps

    def products_scores(p, ch):
        pr = prp.tile([P, 4, 512], bf16, name="pr", tag="pr")
        # offsets (5,3) and (1,0): two stride-uniform 2-offset views
        q_b = q_sb[ch][:, p * 512:(p + 1) * 512].rearrange(
            "a (o s) -> a o s", o=1).broadcast_to([P, 2, 512])
        kA = k_sb[ch][:, p * 512 + PADK - 5:]
        k53 = bass.AP(tensor=kA.tensor, offset=kA.offset,
                      ap=[kA.ap[0], [2, 2], [1, 512]])
        nc.vector.tensor_mul(pr[:, 0:2, :], q_b, k53)
        kB = k_sb[ch][:, p * 512 + PADK - 1:]
        k10 = bass.AP(tensor=kB.tensor, offset=kB.offset,
                      ap=[kB.ap[0], [1, 2], [1, 512]])
        nc.vector.tensor_mul(pr[:, 2:4, :], q_b, k10)
        for ci in range(4):
            # start=True on each range's first MM clears stale has_written
            # in the recycled psum banks
            nc.tensor.matmul(
                sc[:, ci * 512:(ci + 1) * 512],
                lhsT=selkm[ch][:, 96 - 32 * p: 224 - 32 * p],
                rhs=pr[:, ci, :],
                start=(p == 0 and ch == 0),
                stop=(p == 3 and ch == 1))

    # ---------------- interleaved projections + products/scores ------
    sc = scp.tile([P, 4 * 512], f32, name="sc", tag="sc")

    def qq(ch, qi):
        proj_q("xq", "wq", 0, ch, qi,
               q_sb[ch][:, qi * 512:(qi + 1) * 512])

    def kk(ch, qi):
        proj_q("xk", "wk", 1, ch, qi,
               k_sb[ch][:, PADK + qi * 512:PADK + (qi + 1) * 512])

    def vv(ch, qi, drain=True):
        return proj_q("xv", "wv", 2, ch, qi,
                      v_sb[ch][:, PADK + qi * 512:PADK + (qi + 1) * 512]
                      if drain else None, drain=drain)

    for qi in range(4):
        qq(0, qi)
        kk(0, qi)
        products_scores(qi, 0)
    qq(1, 0)
    kk(1, 0)
    products_scores(0, 1)
    vv(0, 0)
    qq(1, 1)
    kk(1, 1)
    products_scores(1, 1)
    vv(0, 1)
    qq(1, 2)
    kk(1, 2)
    products_scores(2, 1)
    vv(0, 2)
    qq(1, 3)
    kk(1, 3)
    products_scores(3, 1)
    vv(0, 3)
    # mask: scores for s_loc < c -> -1e9 (covers every pair block at once)
    for ci, c in enumerate(CONS):
        if c:
            nc.vector.memset(sc[:, ci * 512: ci * 512 + c], NEG)

    # ---------------- softmax (exp halves overlap den adds) ----------
    exp_sb = main.tile([P, 4, 512], bf16, name="exp_sb")
    nc.scalar.activation(
        exp_sb[:, 0:2, :].rearrange("a o s -> a (o s)"), sc[:, 0:1024],
        Act.Exp)
    d1 = main.tile([P, 2, 512], bf16, name="d1")
    nc.vector.tensor_add(d1[:, 0, :], exp_sb[:, 0, :], exp_sb[:, 1, :])
    nc.scalar.activation(
        exp_sb[:, 2:4, :].rearrange("a o s -> a (o s)"), sc[:, 1024:2048],
        Act.Exp)
    nc.vector.tensor_add(d1[:, 1, :], exp_sb[:, 2, :], exp_sb[:, 3, :])
    # v-ch1 matmuls fill the PE gap during the softmax chain; their
    # drains are deferred past p_norm so they don't block exp on ACT
    v_ps = {}
    for qi in range(4):
        v_ps[qi] = vv(1, qi, drain=False)
    den = main.tile([P, 512], f32, name="den")
    nc.vector.tensor_add(den[:], d1[:, 0, :], d1[:, 1, :])
    rcp = main.tile([P, 512], f32, name="rcp")
    nc.vector.reciprocal_approx_fast(rcp[:], den[:])
    rcp_b = main.tile([P, 512], bf16, name="rcp_b")
    nc.vector.tensor_copy(rcp_b[:], rcp[:])
    p_norm = main.tile([P, 4, 512], bf16, name="p_norm")
    rcp_bb = rcp_b[:].rearrange("a (o s) -> a o s", o=1).broadcast_to(
        [P, 2, 512])
    nc.vector.tensor_mul(p_norm[:, 0:2, :], exp_sb[:, 0:2, :], rcp_bb)
    nc.vector.tensor_mul(p_norm[:, 2:4, :], exp_sb[:, 2:4, :], rcp_bb)
    for qi in range(4):
        nc.scalar.activation(
            v_sb[1][:, PADK + qi * 512:PADK + (qi + 1) * 512], v_ps[qi][:],
            Act.Identity, bias=bqkv[1][:, 2:3])

    # close projection/score psum pools; open broadcast + output pools
    scp_cm.__exit__(None, None, None)
    pjh_cm.__exit__(None, None, None)
    bcp = ctx.enter_context(tc.tile_pool(name="bcp", bufs=3, space="PSUM"))
    yp = ctx.enter_context(tc.tile_pool(name="yp", bufs=2, space="PSUM"))

    # ---------------- broadcast p + apply + output projection --------
    y_sb = [main.tile([P, ROWS], bf16, name=f"y_sb{o}") for o in range(2)]

    def emit_y(p, ms):
        # ms[ch][:, 2+j, :] holds the two partial sums (m0+m2, m1+m3);
        # psum accumulation across the 4 matmuls finishes the offset sum
        for och in range(2):
            ypt = yp.tile([P, 512], f32, name="ypt", tag="ypt")
            k = 0
            for dch in range(2):
                for j in range(2):
                    nc.tensor.matmul(
                        ypt[:],
                        lhsT=ws["wo", dch][:, och * P:(och + 1) * P],
                        rhs=ms[dch][:, 2 + j, :],
                        start=(k == 0), stop=(k == 3))
                    k += 1
            if p == NPAIR - 1 and och == 1:
                # tail: run the last drain on DVE, parallel to ACT's och0
                nc.vector.tensor_scalar(
                    y_sb[och][:, p * 512:(p + 1) * 512], ypt[:],
                    bo_sb[:, och:och + 1], None, mybir.AluOpType.add)
            else:
                nc.scalar.activation(
                    y_sb[och][:, p * 512:(p + 1) * 512], ypt[:],
                    Act.Identity, bias=bo_sb[:, och:och + 1])
            nc.sync.dma_start(
                out=y_dram[och * P:(och + 1) * P, p * 512:(p + 1) * 512],
                in_=y_sb[och][:, p * 512:(p + 1) * 512])

    prev = None
    for p in range(NPAIR):
        m_pair = {}
        for ch in range(2):
            # broadcast p back to feature partitions: (5,3) half drained
            # to SBUF by ACT (enables 2x DVE mul), (1,0) half from psum
            bc = bcp.tile([P, 2, 512], f32, name="bc", tag="bc")
            for j in range(2):
                nc.tensor.matmul(
                    bc[:, j, :], lhsT=selmk[p][ch],
                    rhs=p_norm[:, j, :], start=True, stop=True)
            bc2 = bcp.tile([P, 2, 512], f32, name="bc2", tag="bc")
            for j in range(2):
                nc.tensor.matmul(
                    bc2[:, j, :], lhsT=selmk[p][ch],
                    rhs=p_norm[:, 2 + j, :], start=True, stop=True)
            bc_sb = bsp.tile([P, 2, 512], bf16, name="bc_sb", tag="bs")
            nc.scalar.activation(
                bc_sb[:].rearrange("a o s -> a (o s)"),
                bc[:].rearrange("a o s -> a (o s)"), Act.Identity)
            m = mp.tile([P, 4, 512], bf16, name="m", tag="m")
            vA = v_sb[ch][:, p * 512 + PADK - 5:]
            v53 = bass.AP(tensor=vA.tensor, offset=vA.offset,
                          ap=[vA.ap[0], [2, 2], [1, 512]])
            nc.vector.tensor_mul(m[:, 0:2, :], bc_sb[:], v53)
            vB = v_sb[ch][:, p * 512 + PADK - 1:]
            v10 = bass.AP(tensor=vB.tensor, offset=vB.offset,
                          ap=[vB.ap[0], [1, 2], [1, 512]])
            nc.vector.tensor_mul(m[:, 2:4, :], bc2[:], v10)
            if p < NPAIR - 1:
                # halve the offset sum with an in-place accumulate DMA
                # (SDMA CCE add): m[:, 2:4] += m[:, 0:2]
                nc.gpsimd.dma_start(
                    out=m[:, 2:4, :], in_=m[:, 0:2, :],
                    accum_op=mybir.AluOpType.add)
            else:
                # last pair: DVE add keeps the tail off the slow SWDGE path
                nc.vector.tensor_add(m[:, 2:4, :], m[:, 0:2, :], m[:, 2:4, :])
            m_pair[ch] = m
        if prev is not None:
            emit_y(prev[0], prev[1])
        prev = (p, m_pair)
    emit_y(prev[0], prev[1])


def build_nc():
    from contextlib import ExitStack
    nc = bacc.Bacc(trn_type="TRN2", target_bir_lowering=False, debug=False)
    d = {}
    for name in ("xq", "xk", "xv"):
        d[name] = nc.dram_tensor(name, [DM, ROWS], bf16, kind="ExternalInput").ap()
    d["wq"] = nc.dram_tensor("wq", [P, 512], bf16, kind="ExternalInput").ap()
    d["wk"] = nc.dram_tensor("wk", [P, 512], bf16, kind="ExternalInput").ap()
    d["wvo"] = nc.dram_tensor("wvo", [P, 1024], bf16, kind="ExternalInput").ap()
    d["cst"] = nc.dram_tensor("cst", [P, 8], f32, kind="ExternalInput").ap()
    d["selkm"] = nc.dram_tensor("selkm", [P, 448], bf16, kind="ExternalInput").ap()
    d["selmk"] = nc.dram_tensor("selmk", [P, 1024], bf16, kind="ExternalInput").ap()
    y = nc.dram_tensor("y", [DM, ROWS], bf16, kind="ExternalOutput").ap()
    with tile.TileContext(nc) as tc:
        with ExitStack() as ctx:
            _emit(ctx, tc, nc, d, y)
    nc.compile()
    return nc


def _pack_w(W):
    # [256, 256] W.T -> [128, 2, 256] partition-major chunk pack
    wt = np.asarray(W, np.float32).T.astype(BF)
    return np.ascontiguousarray(wt.reshape(2, P, DM).transpose(1, 0, 2)
                                ).reshape(P, 512)


def make_shared_inputs(Wq, bq, Wk, bk, Wv, bv, Wo, bo):
    shared = {}
    shared["wq"] = _pack_w(Wq)
    shared["wk"] = _pack_w(Wk)
    shared["wvo"] = np.concatenate([_pack_w(Wv), _pack_w(Wo)], axis=1)
    bqkv = np.stack([bq, bk, bv], axis=1).astype(np.float32).reshape(2, P, 3)
    cst = np.zeros((P, 8), np.float32)
    cst[:, 0:3] = bqkv[0]
    cst[:, 3:6] = bqkv[1]
    cst[:, 6:8] = np.asarray(bo, np.float32).reshape(2, P).T
    shared["cst"] = cst
    # selkm[ch, d, 96+h] = 0.5 iff h == global head of feature ch*128+d.
    # The score matmul for pair p uses lhsT = selkm[ch][:, 96-32p : 224-32p],
    # whose column j = 32p+h lands the head-h sum on psum partition 32p+h.
    selkm = np.zeros((2, P, 224), np.float32)
    for ch in range(2):
        for dd in range(P):
            selkm[ch, dd, 96 + ch * 4 + dd // 32] = SCALE
    shared["selkm"] = np.ascontiguousarray(
        selkm.transpose(1, 0, 2).reshape(P, 448)).astype(BF)
    # selmk[p, ch, 32p+j, d] = 1 iff global head of feature ch*128+d == j
    # packed as [128 rows j', 8 blocks (2p+ch), 128 cols d] -> lhsT views
    selmk = np.zeros((NPAIR, 2, P, P), np.float32)
    for p in range(NPAIR):
        for ch in range(2):
            for dd in range(P):
                selmk[p, ch, 32 * p + ch * 4 + dd // 32, dd] = 1.0
    shared["selmk"] = np.ascontiguousarray(
        selmk.reshape(NPAIR * 2, P, P).transpose(1, 0, 2).reshape(P, 1024)
    ).astype(BF)
    return shared


def make_core_inputs(query, key_in, value, core):
    # core i handles stock i: [4, 512, 256] -> feature-major [256, 2048]
    out = {}
    for name, x in (("xq", query), ("xk", key_in), ("xv", value)):
        xi = np.asarray(x[core], dtype=np.float32).reshape(ROWS, DM)
        out[name] = np.ascontiguousarray(xi.T).astype(BF)
    return out


def kernel(query, key_in, value, Wq, bq, Wk, bk, Wv, bv, Wo, bo):
    nc = build_nc()
    shared = make_shared_inputs(Wq, bq, Wk, bk, Wv, bv, Wo, bo)
    in_maps = []
    for core in range(NCORES):
        m = dict(shared)
        m.update(make_core_inputs(query, key_in, value, core))
        in_maps.append(m)
    res = run_bass_kernel_spmd(nc, in_maps, list(range(NCORES))).results
    y = np.stack([
        np.asarray(res[i]["y"]).astype(np.float32).T.reshape(NB, S, DM)
        for i in range(NCORES)])
    return y


# revision 26
# speedup vs baseline: 1.0190x; 1.0190x over previous
"""Trainium2 Bass kernel for sparse multi-headed attention (bf16 pipeline).

Semantics (verified against the reference):
  q = x_q @ Wq.T + bq (per head, dk=32), same for k, v
  for each row s: attend to keys {s-c : c in (5,3,1,0), c <= s}
    score_c[s] = q[s].k[s-c] / sqrt(4)
    p = softmax over valid offsets
    attn[s] = sum_c p_c[s] * v[s-c]
  y = attn @ Wo.T + bo

Sharding: data-parallel over d_stock (8 stocks -> 8 cores). Each core
processes 4 (stock,batch) pairs = 2048 rows. Weights replicated.

Device layout: feature-major bf16 activations [256 feats, 2048 rows]
(host pre-transposes + casts). Projections run through [128, 1024]
half-psums (2 banks, ring of 2) so drains pipeline behind the next
half's matmuls, and products/score-matmuls interleave with later
projections. Scores/softmax live in the compact pair-block layout
[128 partitions = 4 pairs x 32, 4 offsets x 512] (fp32 psum -> bf16
sbuf). p is broadcast back to feature partitions with selector
matmuls; the (5,3) halves are drained to SBUF by the scalar engine
(2x DVE muls), the (1,0) halves are multiplied straight out of psum.
Output y is emitted feature-major bf16 and transposed/upcast on host.
k/v tiles carry a 9-col front pad so shifted reads stay 4B-aligned.
"""

import numpy as np
import ml_dtypes

from concourse import bacc, bass, mybir, tile
from concourse.bass_utils import run_bass_kernel_spmd

DS, NB, S, DM, H, DK = 8, 4, 512, 256, 8, 32
CONS = (5, 3, 1, 0)
NCORES = 8
NPAIR = NB  # pairs per core (1 stock x 4 batches)
ROWS = NPAIR * S  # 2048
P = 128
PADK = 9  # front pad columns for k/v (odd -> shifted (5,3,1) reads aligned)
NEG = -1e9
SCALE = 0.5  # 1/sqrt(n_att)

f32 = mybir.dt.float32
bf16 = mybir.dt.bfloat16
Act = mybir.ActivationFunctionType
BF = ml_dtypes.bfloat16


def _emit(ctx, tc, nc, d, y_dram):
    main = ctx.enter_context(tc.tile_pool(name="main", bufs=1))
    prp = ctx.enter_context(tc.tile_pool(name="prp", bufs=3))
    mp = ctx.enter_context(tc.tile_pool(name="mp", bufs=5))
    bsp = ctx.enter_context(tc.tile_pool(name="bsp", bufs=4))
    pjh_cm = tc.tile_pool(name="pjh", bufs=4, space="PSUM")
    pjh = pjh_cm.__enter__()
    scp_cm = tc.tile_pool(name="scp", bufs=1, space="PSUM")
    scp = scp_cm.__enter__()

    # ---------------- PE warmup (HAM un-throttle) while DMAs run ----
    wtile = main.tile([P, 512], bf16, name="wtile")
    nc.vector.memset(wtile[:], 0.0)
    wps = pjh.tile([P, 512], f32, name="pj", tag="pj")
    for i in range(4):
        nc.tensor.matmul(
            wps[:], lhsT=wtile[:, 0:P], rhs=wtile[:], start=True, stop=True)

    # ---------------- loads ----------------
    # sync queue: q-side, v, selectors, weights, y stores.
    # scalar queue: k-side early loads only, then pure ACT work.
    xs = {}
    for name in ("xq", "xk", "xv"):
        for ch in range(2):
            xs[name, ch] = main.tile([P, ROWS], bf16, name=f"{name}{ch}")
    # packed constants: single DMA per group (issue cost ~600ns each,
    # so small tensors ride together)
    wqp = main.tile([P, 512], bf16, name="wqp")
    wkp = main.tile([P, 512], bf16, name="wkp")
    wvop = main.tile([P, 1024], bf16, name="wvop")
    cst = main.tile([P, 8], f32, name="cst")
    skm = main.tile([P, 448], bf16, name="skm")
    smk = main.tile([P, 1024], bf16, name="smk")
    ws = {}
    for ch in range(2):
        ws["wq", ch] = wqp[:, ch * DM:(ch + 1) * DM]
        ws["wk", ch] = wkp[:, ch * DM:(ch + 1) * DM]
        ws["wv", ch] = wvop[:, ch * DM:(ch + 1) * DM]
        ws["wo", ch] = wvop[:, 512 + ch * DM:512 + (ch + 1) * DM]
    bqkv = [cst[:, 0:3], cst[:, 3:6]]
    bo_sb = cst[:, 6:8]
    selkm = [skm[:, 0:224], skm[:, 224:448]]
    selmk = [[smk[:, (2 * p + ch) * P:(2 * p + ch + 1) * P]
              for ch in range(2)] for p in range(NPAIR)]
    nc.sync.dma_start(out=wqp[:], in_=d["wq"])
    nc.scalar.dma_start(out=wkp[:], in_=d["wk"])
    nc.scalar.dma_start(out=cst[:], in_=d["cst"])
    # x loads: pair-0 quarter first (unblocks the first projections),
    # then the remaining 3 pairs in one transfer
    for lo, hi in ((0, 512), (512, 2048)):
        for ch in range(2):
            nc.sync.dma_start(out=xs["xq", ch][:, lo:hi],
                              in_=d["xq"][ch * P:(ch + 1) * P, lo:hi])
            nc.scalar.dma_start(out=xs["xk", ch][:, lo:hi],
                                in_=d["xk"][ch * P:(ch + 1) * P, lo:hi])
    nc.sync.dma_start(out=skm[:], in_=d["selkm"])
    nc.sync.dma_start(out=wvop[:], in_=d["wvo"])
    for lo, hi in ((0, 1024), (1024, 2048)):
        for ch in range(2):
            nc.sync.dma_start(out=xs["xv", ch][:, lo:hi],
                              in_=d["xv"][ch * P:(ch + 1) * P, lo:hi])
    nc.sync.dma_start(out=smk[:], in_=d["selmk"])

    # ---------------- projection tiles ----------------
    # widths kept even so bf16 tiles never knock later SBUF allocations
    # off 4-byte alignment (odd width would disable DVE 2x)
    q_sb = [main.tile([P, ROWS], bf16, name=f"q_sb{ch}") for ch in range(2)]
    k_sb = [main.tile([P, PADK + ROWS + 1], bf16, name=f"k_sb{ch}")
            for ch in range(2)]
    v_sb = [main.tile([P, PADK + ROWS + 1], bf16, name=f"v_sb{ch}")
            for ch in range(2)]
    for ch in range(2):
        nc.vector.memset(k_sb[ch][:, 0:PADK], 0.0)
        nc.vector.memset(v_sb[ch][:, 0:PADK], 0.0)

    def proj_q(name, wname, bcol, ch, qi, out_ap, drain=True):
        """Quarter-projection: rows [512qi, 512qi+512) of output chunk
        ch; ACT drain adds bias and casts bf16."""
        ps = pjh.tile([P, 512], f32, name="pj", tag="pj")
        lo = 512 * qi
        for kch in range(2):
            nc.tensor.matmul(
                ps[:],
                lhsT=ws[wname, kch][:, ch * P:(ch + 1) * P],
                rhs=xs[name, kch][:, lo:lo + 512],
                start=(kch == 0), stop=(kch == 1))
        if drain:
            nc.scalar.activation(
                out_ap, ps[:], Act.Identity, bias=bqkv[ch][:, bcol:bcol + 1])
        return ps

    def products_scores(p, ch):
        pr = prp.tile([P, 4, 512], bf16, name="pr", tag="pr")
        # offsets (5,3) and (1,0): two stride-uniform 2-offset views
        q_b = q_sb[ch][:, p * 512:(p + 1) * 512].rearrange(
            "a (o s) -> a o s", o=1).broadcast_to([P, 2, 512])
        kA = k_sb[ch][:, p * 512 + PADK - 5:]
        k53 = bass.AP(tensor=kA.tensor, offset=kA.offset,
                      ap=[kA.ap[0], [2, 2], [1, 512]])
        nc.vector.tensor_mul(pr[:, 0:2, :], q_b, k53)
        kB = k_sb[ch][:, p * 512 + PADK - 1:]
        k10 = bass.AP(tensor=kB.tensor, offset=kB.offset,
                      ap=[kB.ap[0], [1, 2], [1, 512]])
        nc.vector.tensor_mul(pr[:, 2:4, :], q_b, k10)
        for ci in range(4):
            # start=True on each range's first MM clears stale has_written
            # in the recycled psum banks
            nc.tensor.matmul(
                sc[:, ci * 512:(ci + 1) * 512],
                lhsT=selkm[ch][:, 96 - 32 * p: 224 - 32 * p],
                rhs=pr[:, ci, :],
                start=(p == 0 and ch == 0),
                stop=(p == 3 and ch == 1))

    # ---------------- interleaved projections + products/scores ------
    sc = scp.tile([P, 4 * 512], f32, name="sc", tag="sc")

    def qq(ch, qi):
        proj_q("xq", "wq", 0, ch, qi,
               q_sb[ch][:, qi * 512:(qi + 1) * 512])

    def kk(ch, qi):
        proj_q("xk", "wk", 1, ch, qi,
               k_sb[ch][:, PADK + qi * 512:PADK + (qi + 1) * 512])

    def vv(ch, qi, drain=True):
        return proj_q("xv", "wv", 2, ch, qi,
                      v_sb[ch][:, PADK + qi * 512:PADK + (qi + 1) * 512]
                      if drain else None, drain=drain)

    for qi in range(4):
        qq(0, qi)
        kk(0, qi)
        products_scores(qi, 0)
    qq(1, 0)
    kk(1, 0)
    products_scores(0, 1)
    vv(0, 0)
    qq(1, 1)
    kk(1, 1)
    products_scores(1, 1)
    vv(0, 1)
    qq(1, 2)
    kk(1, 2)
    products_scores(2, 1)
    vv(0, 2)
    qq(1, 3)
    kk(1, 3)
    products_scores(3, 1)
    vv(0, 3)
    # mask: scores for s_loc < c -> -1e9 (covers every pair block at once)
    for ci, c in enumerate(CONS):
        if c:
            nc.vector.memset(sc[:, ci * 512: ci * 512 + c], NEG)

    # ---------------- softmax (exp halves overlap den adds) ----------
    exp_sb = main.tile([P, 4, 512], bf16, name="exp_sb")
    nc.scalar.activation(
        exp_sb[:, 0:2, :].rearrange("a o s -> a (o s)"), sc[:, 0:1024],
        Act.Exp)
    d1 = main.tile([P, 2, 512], bf16, name="d1")
    nc.vector.tensor_add(d1[:, 0, :], exp_sb[:, 0, :], exp_sb[:, 1, :])
    nc.scalar.activation(
        exp_sb[:, 2:4, :].rearrange("a o s -> a (o s)"), sc[:, 1024:2048],
        Act.Exp)
    nc.vector.tensor_add(d1[:, 1, :], exp_sb[:, 2, :], exp_sb[:, 3, :])
    # v-ch1 matmuls fill the PE gap during the softmax chain; their
    # drains are deferred past p_norm so they don't block exp on ACT
    v_ps = {}
    for qi in range(4):
        v_ps[qi] = vv(1, qi, drain=False)
    den = main.tile([P, 512], f32, name="den")
    nc.vector.tensor_add(den[:], d1[:, 0, :], d1[:, 1, :])
    rcp = main.tile([P, 512], f32, name="rcp")
    nc.vector.reciprocal_approx_fast(rcp[:], den[:])
    rcp_b = main.tile([P, 512], bf16, name="rcp_b")
    nc.vector.tensor_copy(rcp_b[:], rcp[:])
    p_norm = main.tile([P, 4, 512], bf16, name="p_norm")
    rcp_bb = rcp_b[:].rearrange("a (o s) -> a o s", o=1).broadcast_to(
        [P, 2, 512])
    nc.vector.tensor_mul(p_norm[:, 0:2, :], exp_sb[:, 0:2, :], rcp_bb)
    nc.vector.tensor_mul(p_norm[:, 2:4, :], exp_sb[:, 2:4, :], rcp_bb)
    for qi in range(4):
        nc.scalar.activation(
            v_sb[1][:, PADK + qi * 512:PADK + (qi + 1) * 512], v_ps[qi][:],
            Act.Identity, bias=bqkv[1][:, 2:3])

    # close projection/score psum pools; open broadcast + output pools
    scp_cm.__exit__(None, None, None)
    pjh_cm.__exit__(None, None, None)
    bcp = ctx.enter_context(tc.tile_pool(name="bcp", bufs=3, space="PSUM"))
    yp = ctx.enter_context(tc.tile_pool(name="yp", bufs=2, space="PSUM"))

    # ---------------- broadcast p + apply + output projection --------
    y_sb = [main.tile([P, ROWS], bf16, name=f"y_sb{o}") for o in range(2)]

    def emit_y(p, ms):
        # ms[ch][:, 2+j, :] holds the two partial sums (m0+m2, m1+m3);
        # psum accumulation across the 4 matmuls finishes the offset sum
        for och in range(2):
            ypt = yp.tile([P, 512], f32, name="ypt", tag="ypt")
            k = 0
            for dch in range(2):
                for j in range(2):
                    nc.tensor.matmul(
                        ypt[:],
                        lhsT=ws["wo", dch][:, och * P:(och + 1) * P],
                        rhs=ms[dch][:, 2 + j, :],
                        start=(k == 0), stop=(k == 3))
                    k += 1
            if p == NPAIR - 1 and och == 1:
                # tail: run the last drain on DVE, parallel to ACT's och0
                nc.vector.tensor_scalar(
                    y_sb[och][:, p * 512:(p + 1) * 512], ypt[:],
                    bo_sb[:, och:och + 1], None, mybir.AluOpType.add)
            else:
                nc.scalar.activation(
                    y_sb[och][:, p * 512:(p + 1) * 512], ypt[:],
                    Act.Identity, bias=bo_sb[:, och:och + 1])
            nc.sync.dma_start(
                out=y_dram[och * P:(och + 1) * P, p * 512:(p + 1) * 512],
                in_=y_sb[och][:, p * 512:(p + 1) * 512])

    prev = None
    for p in range(NPAIR):
        m_pair = {}
        for ch in range(2):
            # broadcast p back to feature partitions: (5,3) half drained
            # to SBUF by ACT (enables 2x DVE mul), (1,0) half from psum
            bc = bcp.tile([P, 2, 512], f32, name="bc", tag="bc")
            for j in range(2):
                nc.tensor.matmul(
                    bc[:, j, :], lhsT=selmk[p][ch],
                    rhs=p_norm[:, j, :], start=True, stop=True)
            bc2 = bcp.tile([P, 2, 512], f32, name="bc2", tag="bc")
            for j in range(2):
                nc.tensor.matmul(
                    bc2[:, j, :], lhsT=selmk[p][ch],
                    rhs=p_norm[:, 2 + j, :], start=True, stop=True)
            bc_sb = bsp.tile([P, 2, 512], bf16, name="bc_sb", tag="bs")
            nc.scalar.activation(
                bc_sb[:].rearrange("a o s -> a (o s)"),
                bc[:].rearrange("a o s -> a (o s)"), Act.Identity)
            m = mp.tile([P, 4, 512], bf16, name="m", tag="m")
            vA = v_sb[ch][:, p * 512 + PADK - 5:]
            v53 = bass.AP(tensor=vA.tensor, offset=vA.offset,
                          ap=[vA.ap[0], [2, 2], [1, 512]])
            nc.vector.tensor_mul(m[:, 0:2, :], bc_sb[:], v53)
            vB = v_sb[ch][:, p * 512 + PADK - 1:]
            v10 = bass.AP(tensor=vB.tensor, offset=vB.offset,
                          ap=[vB.ap[0], [1, 2], [1, 512]])
            if ch == 0:
                # ACT has idle gaps here: drain the (1,0) half too so
                # this DVE mul runs at 2x instead of 1x-from-psum
                bc2_sb = bsp.tile([P, 2, 512], bf16, name="bc2_sb", tag="bs")
                nc.scalar.activation(
                    bc2_sb[:].rearrange("a o s -> a (o s)"),
                    bc2[:].rearrange("a o s -> a (o s)"), Act.Identity)
                nc.vector.tensor_mul(m[:, 2:4, :], bc2_sb[:], v10)
            else:
                nc.vector.tensor_mul(m[:, 2:4, :], bc2[:], v10)
            if p < NPAIR - 1:
                # halve the offset sum with an in-place accumulate DMA
                # (SDMA CCE add): m[:, 2:4] += m[:, 0:2]
                nc.gpsimd.dma_start(
                    out=m[:, 2:4, :], in_=m[:, 0:2, :],
                    accum_op=mybir.AluOpType.add)
            else:
                # last pair: DVE add keeps the tail off the slow SWDGE path
                nc.vector.tensor_add(m[:, 2:4, :], m[:, 0:2, :], m[:, 2:4, :])
            m_pair[ch] = m
        if prev is not None:
            emit_y(prev[0], prev[1])
        prev = (p, m_pair)
    emit_y(prev[0], prev[1])


def build_nc():
    from contextlib import ExitStack
    nc = bacc.Bacc(trn_type="TRN2", target_bir_lowering=False, debug=False)
    d = {}
    for name in ("xq", "xk", "xv"):
        d[name] = nc.dram_tensor(name, [DM, ROWS], bf16, kind="ExternalInput").ap()
    d["wq"] = nc.dram_tensor("wq", [P, 512], bf16, kind="ExternalInput").ap()
    d["wk"] = nc.dram_tensor("wk", [P, 512], bf16, kind="ExternalInput").ap()
    d["wvo"] = nc.dram_tensor("wvo", [P, 1024], bf16, kind="ExternalInput").ap()
    d["cst"] = nc.dram_tensor("cst", [P, 8], f32, kind="ExternalInput").ap()
    d["selkm"] = nc.dram_tensor("selkm", [P, 448], bf16, kind="ExternalInput").ap()
    d["selmk"] = nc.dram_tensor("selmk", [P, 1024], bf16, kind="ExternalInput").ap()
    y = nc.dram_tensor("y", [DM, ROWS], bf16, kind="ExternalOutput").ap()
    with tile.TileContext(nc) as tc:
        with ExitStack() as ctx:
            _emit(ctx, tc, nc, d, y)
    nc.compile()
    return nc


def _pack_w(W):
    # [256, 256] W.T -> [128, 2, 256] partition-major chunk pack
    wt = np.asarray(W, np.float32).T.astype(BF)
    return np.ascontiguousarray(wt.reshape(2, P, DM).transpose(1, 0, 2)
                                ).reshape(P, 512)


def make_shared_inputs(Wq, bq, Wk, bk, Wv, bv, Wo, bo):
    shared = {}
    shared["wq"] = _pack_w(Wq)
    shared["wk"] = _pack_w(Wk)
    shared["wvo"] = np.concatenate([_pack_w(Wv), _pack_w(Wo)], axis=1)
    bqkv = np.stack([bq, bk, bv], axis=1).astype(np.float32).reshape(2, P, 3)
    cst = np.zeros((P, 8), np.float32)
    cst[:, 0:3] = bqkv[0]
    cst[:, 3:6] = bqkv[1]
    cst[:, 6:8] = np.asarray(bo, np.float32).reshape(2, P).T
    shared["cst"] = cst
    # selkm[ch, d, 96+h] = 0.5 iff h == global head of feature ch*128+d.
    # The score matmul for pair p uses lhsT = selkm[ch][:, 96-32p : 224-32p],
    # whose column j = 32p+h lands the head-h sum on psum partition 32p+h.
    selkm = np.zeros((2, P, 224), np.float32)
    for ch in range(2):
        for dd in range(P):
            selkm[ch, dd, 96 + ch * 4 + dd // 32] = SCALE
    shared["selkm"] = np.ascontiguousarray(
        selkm.transpose(1, 0, 2).reshape(P, 448)).astype(BF)
    # selmk[p, ch, 32p+j, d] = 1 iff global head of feature ch*128+d == j
    # packed as [128 rows j', 8 blocks (2p+ch), 128 cols d] -> lhsT views
    selmk = np.zeros((NPAIR, 2, P, P), np.float32)
    for p in range(NPAIR):
        for ch in range(2):
            for dd in range(P):
                selmk[p, ch, 32 * p + ch * 4 + dd // 32, dd] = 1.0
    shared["selmk"] = np.ascontiguousarray(
        selmk.reshape(NPAIR * 2, P, P).transpose(1, 0, 2).reshape(P, 1024)
    ).astype(BF)
    return shared


def make_core_inputs(query, key_in, value, core):
    # core i handles stock i: [4, 512, 256] -> feature-major [256, 2048]
    out = {}
    for name, x in (("xq", query), ("xk", key_in), ("xv", value)):
        xi = np.asarray(x[core], dtype=np.float32).reshape(ROWS, DM)
        out[name] = np.ascontiguousarray(xi.T).astype(BF)
    return out


def kernel(query, key_in, value, Wq, bq, Wk, bk, Wv, bv, Wo, bo):
    nc = build_nc()
    shared = make_shared_inputs(Wq, bq, Wk, bk, Wv, bv, Wo, bo)
    in_maps = []
    for core in range(NCORES):
        m = dict(shared)
        m.update(make_core_inputs(query, key_in, value, core))
        in_maps.append(m)
    res = run_bass_kernel_spmd(nc, in_maps, list(range(NCORES))).results
    y = np.stack([
        np.asarray(res[i]["y"]).astype(np.float32).T.reshape(NB, S, DM)
        for i in range(NCORES)])
    return y
